# revision 16
# baseline (speedup 1.0000x reference)
"""TRN2 Bass/Tile kernel: 16-head MHA, B=1 S=4096 E=1024, head-sharded over 8 cores.

Sharding: tensor-parallel over heads. Core c owns heads {2c, 2c+1}: columns
[128c, 128(c+1)) of Wq/Wk/Wv (+bias slices) and rows [128c, 128(c+1)) of Wo.
Each core computes attention for its 2 heads and a partial out-projection
[S, E]; the host sums the 8 partials and adds bo (TP row-parallel unshard).

Per-core pipeline (all matmuls float32r, fp32 PSUM accumulate):
  A) QT/KT/VT [128ch, S] = W_c^T @ x^T   (lhsT=W-slice, rhs=xT tiles, +bias)
  B) V natural [S,ch] via PE transpose, packed [V_h|ones] for the l-sum trick
  C) scores^T [k,q] = K_h @ Q_h^T -> exp on ACT (scale=1/8) -> PV accumulate
     psum[65, q]: rows 0:64 = unnormalized attn^T, row 64 = softmax denom l
  D) recip(l) -> DMA partition-broadcast -> scale attn^T -> out-proj @ Wo_c
"""

import sys

for _p in ("/opt/trn_rl_repo", "/opt/pypackages"):
    if _p not in sys.path:
        sys.path.append(_p)

import numpy as np

EMBED = 1024
N_CORES = 8
HC = EMBED // N_CORES  # 128 channels = 2 heads per core
DH = 64                # head dim
SEQ = 4096

_NC_CACHE = {}


def _build_nc(S=SEQ, E=EMBED, mmdt="fp16"):
    from contextlib import ExitStack

    import concourse.bass as bass
    import concourse.mybir as mybir
    import concourse.tile as tile
    from concourse import bacc
    from concourse.masks import make_identity

    F32 = mybir.dt.float32
    MMDT = {"fp16": mybir.dt.float16, "f32r": mybir.dt.float32r,
            "fp32": mybir.dt.float32}[mmdt]

    ET = E // 128      # E-tiles of 128 (contraction for projections)
    NSC = S // 512     # 512-wide S chunks
    NKT = S // 128     # 128-wide key tiles
    KTG = 2            # key tiles per exp group ([128, 1024] psum staging)
    NG = NKT // KTG
    NQS = 512 // 128   # 128-q subtiles per chunk
    NEC = E // 512     # 512-wide E chunks of the out-projection

    def mm(ap):
        return ap

    nc = bacc.Bacc()
    xT = nc.declare_dram_parameter("xT", [E, S], MMDT, isOutput=False)
    wq = nc.declare_dram_parameter("wq", [E, HC], MMDT, isOutput=False)
    wk = nc.declare_dram_parameter("wk", [E, HC], MMDT, isOutput=False)
    wv = nc.declare_dram_parameter("wv", [E, HC], MMDT, isOutput=False)
    bq = nc.declare_dram_parameter("bq", [HC, 1], F32, isOutput=False)
    bk = nc.declare_dram_parameter("bk", [HC, 1], F32, isOutput=False)
    bv = nc.declare_dram_parameter("bv", [HC, 1], F32, isOutput=False)
    wo = nc.declare_dram_parameter("wo", [HC, E], MMDT, isOutput=False)
    out = nc.declare_dram_parameter("out", [S, E], F32, isOutput=True)

    with tile.TileContext(nc) as tc, ExitStack() as ctx:
        wpool = ctx.enter_context(tc.tile_pool(name="w", bufs=1))
        xpool = ctx.enter_context(tc.tile_pool(name="x", bufs=4))
        qkvpool = ctx.enter_context(tc.tile_pool(name="qkv", bufs=1))
        v2pool = ctx.enter_context(tc.tile_pool(name="v2", bufs=1))
        epool = ctx.enter_context(tc.tile_pool(name="e", bufs=2))
        apool = ctx.enter_context(tc.tile_pool(name="a", bufs=3))
        rpool = ctx.enter_context(tc.tile_pool(name="r", bufs=4))
        dpool = ctx.enter_context(tc.tile_pool(name="d", bufs=4, space="DRAM"))
        # PSUM: 4 banks staging + 2 PV accumulators + 2 out/transpose = 8
        spsum = ctx.enter_context(tc.tile_pool(name="sp", bufs=2, space="PSUM"))
        pvpsum = ctx.enter_context(tc.tile_pool(name="pv", bufs=2, space="PSUM"))
        opsum = ctx.enter_context(tc.tile_pool(name="op", bufs=2, space="PSUM"))

        # --- weights / constants ---
        w_sb = {}
        for name, src in (("wq", wq), ("wk", wk), ("wv", wv)):
            t = wpool.tile([128, ET, HC], MMDT, tag=name, name=name)
            nc.sync.dma_start(out=t, in_=src.rearrange("(a p) c -> p a c", p=128))
            w_sb[name] = t
        wo_sb = wpool.tile([HC, E], MMDT, tag="wo")
        nc.sync.dma_start(out=wo_sb, in_=wo[:, :])
        b_sb = {}
        for name, src in (("bq", bq), ("bk", bk), ("bv", bv)):
            t = wpool.tile([HC, 1], F32, tag=name, name=name)
            nc.sync.dma_start(out=t, in_=src[:, :])
            b_sb[name] = t
        ident = wpool.tile([128, 128], MMDT, tag="ident")
        make_identity(nc, ident)
        ones_sb = wpool.tile([128, 1], F32, tag="ones")
        nc.vector.memset(ones_sb, 1.0)

        # --- stage A: QT/KT/VT [128ch, S] chunked by 512 ---
        QT = [qkvpool.tile([HC, 512], MMDT, tag=f"qt{i}", name=f"qt{i}") for i in range(NSC)]
        KT = [qkvpool.tile([HC, 512], MMDT, tag=f"kt{i}", name=f"kt{i}") for i in range(NSC)]
        VT = [qkvpool.tile([HC, 512], MMDT, tag=f"vt{i}", name=f"vt{i}") for i in range(NSC)]
        for sc in range(NSC):
            big1 = spsum.tile([128, 1024], F32, tag="big")
            big2 = opsum.tile([128, 512], F32, tag="pt_po")
            for et in range(ET):
                xt = xpool.tile([128, 512], MMDT, tag="xt")
                nc.sync.dma_start(
                    out=xt, in_=xT[et * 128:(et + 1) * 128, sc * 512:(sc + 1) * 512]
                )
                first, last = et == 0, et == ET - 1
                nc.tensor.matmul(big1[:, 0:512], lhsT=mm(w_sb["wq"][:, et, :]),
                                 rhs=mm(xt), start=first, stop=last)
                nc.tensor.matmul(big1[:, 512:1024], lhsT=mm(w_sb["wk"][:, et, :]),
                                 rhs=mm(xt), start=first, stop=last)
                nc.tensor.matmul(big2[:, 0:512], lhsT=mm(w_sb["wv"][:, et, :]),
                                 rhs=mm(xt), start=first, stop=last)
            nc.vector.tensor_scalar_add(QT[sc], big1[:, 0:512], b_sb["bq"])
            nc.vector.tensor_scalar_add(KT[sc], big1[:, 512:1024], b_sb["bk"])
            nc.vector.tensor_scalar_add(VT[sc], big2[:, 0:512], b_sb["bv"])

        # --- stage B: V2 [128k, NKT, 65*2] = [V_h0|ones|V_h1|ones] ---
        V2 = v2pool.tile([128, NKT, 130], MMDT, tag="V2")
        for kt in range(NKT):
            nc.vector.tensor_copy(V2[:, kt, 64:65], ones_sb)
            nc.vector.tensor_copy(V2[:, kt, 129:130], ones_sb)
            pt = opsum.tile([128, 512], MMDT, tag="pt_po")
            nc.tensor.transpose(
                pt[:, 0:128], VT[kt // 4][:, (kt % 4) * 128:(kt % 4 + 1) * 128], ident
            )
            nc.vector.tensor_copy(V2[:, kt, 0:64], pt[:, 0:64])
            nc.vector.tensor_copy(V2[:, kt, 65:129], pt[:, 64:128])

        # --- stages C+D: kt-outer over 1024-q blocks; attnT halves stashed ---
        ATT = [apool.tile([128, 512], MMDT, tag=f"att{i}", name=f"att{i}")
               for i in range(NSC)]
        QBC = 2 if NSC % 2 == 0 else 1  # q-chunks per block
        for h in range(2):
            hs = slice(h * DH, (h + 1) * DH)
            for qb in range(NSC // QBC):
                pvs = [pvpsum.tile([65, 512], F32, tag="pv", name="pv")
                       for _ in range(QBC)]
                for kt in range(NKT):
                    sb = spsum.tile([128, 1024], F32, tag="big")
                    for qc in range(QBC):
                        qq = qb * QBC + qc
                        nc.tensor.matmul(
                            sb[:, qc * 512:(qc + 1) * 512],
                            lhsT=KT[kt // 4][hs, (kt % 4) * 128:(kt % 4 + 1) * 128],
                            rhs=QT[qq][hs, :],
                            start=True, stop=True,
                        )
                    ex = epool.tile([128, 1024], MMDT, tag="ex")
                    nc.scalar.activation(
                        ex[:, 0:QBC * 512], sb[:, 0:QBC * 512],
                        mybir.ActivationFunctionType.Exp, scale=0.125,
                    )
                    for qc in range(QBC):
                        nc.tensor.matmul(
                            pvs[qc],
                            lhsT=V2[:, kt, h * 65:(h + 1) * 65],
                            rhs=ex[:, qc * 512:(qc + 1) * 512],
                            start=(kt == 0), stop=(kt == NKT - 1),
                        )
                # normalize into the stashed attnT half; project after h1
                for qc in range(QBC):
                    qq = qb * QBC + qc
                    rc = rpool.tile([1, 512], F32, tag="rc")
                    nc.vector.reciprocal(rc, pvs[qc][64:65, :])
                    scr = dpool.tile([1, 512], F32, tag="scr")
                    nc.sync.dma_start(out=scr, in_=rc)
                    bc = rpool.tile([DH, 512], F32, tag="bc")
                    nc.sync.dma_start(
                        out=bc,
                        in_=bass.AP(tensor=scr.tensor, offset=scr.offset,
                                    ap=[[0, DH]] + list(scr.ap)[1:]),
                    )
                    nc.vector.tensor_mul(ATT[qq][hs, :], pvs[qc][0:DH, :], bc)
                    if h == 1:
                        for qs in range(NQS):
                            for ec in range(NEC):
                                po = opsum.tile([128, 512], F32, tag="pt_po")
                                nc.tensor.matmul(
                                    po,
                                    lhsT=ATT[qq][:, qs * 128:(qs + 1) * 128],
                                    rhs=wo_sb[:, ec * 512:(ec + 1) * 512],
                                    start=True, stop=True,
                                )
                                osb = apool.tile([128, 512], F32, tag="osb")
                                nc.vector.tensor_copy(osb, po)
                                nc.sync.dma_start(
                                    out=out[qq * 512 + qs * 128:
                                            qq * 512 + (qs + 1) * 128,
                                            ec * 512:(ec + 1) * 512],
                                    in_=osb,
                                )
    nc.finalize()
    return nc


def _get_nc(S=SEQ, mmdt="fp16"):
    key = (S, mmdt)
    if key not in _NC_CACHE:
        _NC_CACHE[key] = _build_nc(S=S, mmdt=mmdt)
    return _NC_CACHE[key]


def _make_in_maps(x, Wq, bq, Wk, bk, Wv, bv, Wo, npdt=np.float16):
    xT = np.ascontiguousarray(np.asarray(x, np.float32)[0].T.astype(npdt))
    Wq, Wk, Wv, Wo = (np.asarray(a, np.float32).astype(npdt) for a in (Wq, Wk, Wv, Wo))
    bq, bk, bv = (np.asarray(a, np.float32) for a in (bq, bk, bv))
    in_maps = []
    for c in range(N_CORES):
        sl = slice(c * HC, (c + 1) * HC)
        in_maps.append({
            "xT": xT,
            "wq": np.ascontiguousarray(Wq[:, sl]),
            "wk": np.ascontiguousarray(Wk[:, sl]),
            "wv": np.ascontiguousarray(Wv[:, sl]),
            "bq": np.ascontiguousarray(bq[sl]).reshape(HC, 1),
            "bk": np.ascontiguousarray(bk[sl]).reshape(HC, 1),
            "bv": np.ascontiguousarray(bv[sl]).reshape(HC, 1),
            "wo": np.ascontiguousarray(Wo[sl, :]),
        })
    return in_maps


def run(inputs, trace=False, mmdt="fp16"):
    """Run the kernel; returns (out [1,S,E] float32, BassKernelResults)."""
    from concourse.bass_utils import run_bass_kernel_spmd

    nc = _get_nc(mmdt=mmdt)
    npdt = np.float16 if mmdt == "fp16" else np.float32
    in_maps = _make_in_maps(
        inputs["x"], inputs["Wq"], inputs["bq"], inputs["Wk"], inputs["bk"],
        inputs["Wv"], inputs["bv"], inputs["Wo"], npdt=npdt,
    )
    res = run_bass_kernel_spmd(
        nc, in_maps, core_ids=list(range(N_CORES)), trace=trace
    )
    acc = np.zeros((SEQ, EMBED), np.float64)
    for c in range(N_CORES):
        acc += res.results[c]["out"]
    acc += np.asarray(inputs["bo"], np.float64)
    return acc.astype(np.float32).reshape(1, SEQ, EMBED), res


def kernel(x, Wq, bq, Wk, bk, Wv, bv, Wo, bo):
    out, _ = run(dict(x=x, Wq=Wq, bq=bq, Wk=Wk, bk=bk, Wv=Wv, bv=bv, Wo=Wo, bo=bo))
    return out


# revision 17
# speedup vs baseline: 1.3804x; 1.3804x over previous
"""TRN2 Bass/Tile kernel: 16-head MHA, B=1 S=4096 E=1024, head-sharded over 8 cores.

Sharding: tensor-parallel over heads. Core c owns heads {2c, 2c+1}: columns
[128c, 128(c+1)) of Wq/Wk/Wv (+bias slices) and rows [128c, 128(c+1)) of Wo.
Each core computes attention for its 2 heads and a partial out-projection
[S, E]; the host sums the 8 partials and adds bo (TP row-parallel unshard).

Per-core pipeline (all matmuls float32r, fp32 PSUM accumulate):
  A) QT/KT/VT [128ch, S] = W_c^T @ x^T   (lhsT=W-slice, rhs=xT tiles, +bias)
  B) V natural [S,ch] via PE transpose, packed [V_h|ones] for the l-sum trick
  C) scores^T [k,q] = K_h @ Q_h^T -> exp on ACT (scale=1/8) -> PV accumulate
     psum[65, q]: rows 0:64 = unnormalized attn^T, row 64 = softmax denom l
  D) recip(l) -> DMA partition-broadcast -> scale attn^T -> out-proj @ Wo_c
"""

import sys

for _p in ("/opt/trn_rl_repo", "/opt/pypackages"):
    if _p not in sys.path:
        sys.path.append(_p)

import numpy as np

EMBED = 1024
N_CORES = 8
HC = EMBED // N_CORES  # 128 channels = 2 heads per core
DH = 64                # head dim
SEQ = 4096

_NC_CACHE = {}


def _build_nc(S=SEQ, E=EMBED, mmdt="fp16"):
    from contextlib import ExitStack

    import concourse.bass as bass
    import concourse.mybir as mybir
    import concourse.tile as tile
    from concourse import bacc
    from concourse.masks import make_identity

    F32 = mybir.dt.float32
    MMDT = {"fp16": mybir.dt.float16, "f32r": mybir.dt.float32r,
            "fp32": mybir.dt.float32}[mmdt]

    ET = E // 128      # E-tiles of 128 (contraction for projections)
    NSC = S // 512     # 512-wide S chunks
    NKT = S // 128     # 128-wide key tiles
    KTG = 2            # key tiles per exp group ([128, 1024] psum staging)
    NG = NKT // KTG
    NQS = 512 // 128   # 128-q subtiles per chunk
    NEC = E // 512     # 512-wide E chunks of the out-projection

    def mm(ap):
        return ap

    nc = bacc.Bacc()
    xT = nc.declare_dram_parameter("xT", [E, S], MMDT, isOutput=False)
    wq = nc.declare_dram_parameter("wq", [E, HC], MMDT, isOutput=False)
    wk = nc.declare_dram_parameter("wk", [E, HC], MMDT, isOutput=False)
    wv = nc.declare_dram_parameter("wv", [E, HC], MMDT, isOutput=False)
    bq = nc.declare_dram_parameter("bq", [HC, 1], F32, isOutput=False)
    bk = nc.declare_dram_parameter("bk", [HC, 1], F32, isOutput=False)
    bv = nc.declare_dram_parameter("bv", [HC, 1], F32, isOutput=False)
    wo = nc.declare_dram_parameter("wo", [HC, E], MMDT, isOutput=False)
    out = nc.declare_dram_parameter("out", [S, E], F32, isOutput=True)

    with tile.TileContext(nc) as tc, ExitStack() as ctx:
        wpool = ctx.enter_context(tc.tile_pool(name="w", bufs=1))
        xpool = ctx.enter_context(tc.tile_pool(name="x", bufs=4))
        qkvpool = ctx.enter_context(tc.tile_pool(name="qkv", bufs=1))
        v2pool = ctx.enter_context(tc.tile_pool(name="v2", bufs=1))
        epool = ctx.enter_context(tc.tile_pool(name="e", bufs=3))
        apool = ctx.enter_context(tc.tile_pool(name="a", bufs=3))
        rpool = ctx.enter_context(tc.tile_pool(name="r", bufs=4))
        dpool = ctx.enter_context(tc.tile_pool(name="d", bufs=4, space="DRAM"))
        # PSUM: 4 banks staging + 2 PV accumulators + 2 out/transpose = 8
        spsum = ctx.enter_context(tc.tile_pool(name="sp", bufs=2, space="PSUM"))
        pvpsum = ctx.enter_context(tc.tile_pool(name="pv", bufs=2, space="PSUM"))
        opsum = ctx.enter_context(tc.tile_pool(name="op", bufs=2, space="PSUM"))

        # --- weights / constants ---
        w_sb = {}
        for name, src in (("wq", wq), ("wk", wk), ("wv", wv)):
            t = wpool.tile([128, ET, HC], MMDT, tag=name, name=name)
            nc.sync.dma_start(out=t, in_=src.rearrange("(a p) c -> p a c", p=128))
            w_sb[name] = t
        wo_sb = wpool.tile([HC, E], MMDT, tag="wo")
        nc.sync.dma_start(out=wo_sb, in_=wo[:, :])
        b_sb = {}
        for name, src in (("bq", bq), ("bk", bk), ("bv", bv)):
            t = wpool.tile([HC, 1], F32, tag=name, name=name)
            nc.sync.dma_start(out=t, in_=src[:, :])
            b_sb[name] = t
        ident = wpool.tile([128, 128], MMDT, tag="ident")
        make_identity(nc, ident)
        ones_sb = wpool.tile([128, 1], F32, tag="ones")
        nc.vector.memset(ones_sb, 1.0)

        # --- stage A: QT/KT/VT [128ch, S] chunked by 512 ---
        QT = [qkvpool.tile([HC, 512], MMDT, tag=f"qt{i}", name=f"qt{i}") for i in range(NSC)]
        KT = [qkvpool.tile([HC, 512], MMDT, tag=f"kt{i}", name=f"kt{i}") for i in range(NSC)]
        VT = [qkvpool.tile([HC, 512], MMDT, tag=f"vt{i}", name=f"vt{i}") for i in range(NSC)]
        for sc in range(NSC):
            big1 = spsum.tile([128, 1024], F32, tag="big")
            big2 = opsum.tile([128, 512], F32, tag="pt_po")
            for et in range(ET):
                xt = xpool.tile([128, 512], MMDT, tag="xt")
                nc.sync.dma_start(
                    out=xt, in_=xT[et * 128:(et + 1) * 128, sc * 512:(sc + 1) * 512]
                )
                first, last = et == 0, et == ET - 1
                nc.tensor.matmul(big1[:, 0:512], lhsT=mm(w_sb["wq"][:, et, :]),
                                 rhs=mm(xt), start=first, stop=last)
                nc.tensor.matmul(big1[:, 512:1024], lhsT=mm(w_sb["wk"][:, et, :]),
                                 rhs=mm(xt), start=first, stop=last)
                nc.tensor.matmul(big2[:, 0:512], lhsT=mm(w_sb["wv"][:, et, :]),
                                 rhs=mm(xt), start=first, stop=last)
            nc.vector.tensor_scalar_add(QT[sc], big1[:, 0:512], b_sb["bq"])
            nc.vector.tensor_scalar_add(KT[sc], big1[:, 512:1024], b_sb["bk"])
            nc.vector.tensor_scalar_add(VT[sc], big2[:, 0:512], b_sb["bv"])

        # --- stage B: V2 [128k, NKT, 65*2] = [V_h0|ones|V_h1|ones] ---
        V2 = v2pool.tile([128, NKT, 130], MMDT, tag="V2")
        for kt in range(NKT):
            nc.vector.tensor_copy(V2[:, kt, 64:65], ones_sb)
            nc.vector.tensor_copy(V2[:, kt, 129:130], ones_sb)
            pt = opsum.tile([128, 512], MMDT, tag="pt_po")
            nc.tensor.transpose(
                pt[:, 0:128], VT[kt // 4][:, (kt % 4) * 128:(kt % 4 + 1) * 128], ident
            )
            nc.vector.tensor_copy(V2[:, kt, 0:64], pt[:, 0:64])
            nc.vector.tensor_copy(V2[:, kt, 65:129], pt[:, 64:128])

        # --- stages C+D: kt-outer over 1024-q blocks; attnT halves stashed ---
        ATT = [apool.tile([128, 512], MMDT, tag=f"att{i}", name=f"att{i}")
               for i in range(NSC)]
        QBC = 2 if NSC % 2 == 0 else 1  # q-chunks per block
        for h in range(2):
            hs = slice(h * DH, (h + 1) * DH)
            for qb in range(NSC // QBC):
                pvs = [pvpsum.tile([65, 512], F32, tag="pv", name="pv")
                       for _ in range(QBC)]
                for kt in range(NKT):
                    sb = spsum.tile([128, 1024], F32, tag="big")
                    for qc in range(QBC):
                        qq = qb * QBC + qc
                        nc.tensor.matmul(
                            sb[:, qc * 512:(qc + 1) * 512],
                            lhsT=KT[kt // 4][hs, (kt % 4) * 128:(kt % 4 + 1) * 128],
                            rhs=QT[qq][hs, :],
                            start=True, stop=True,
                        )
                    ex = epool.tile([128, 1024], MMDT, tag="ex")
                    nc.scalar.activation(
                        ex[:, 0:QBC * 512], sb[:, 0:QBC * 512],
                        mybir.ActivationFunctionType.Exp, scale=0.125,
                    )
                    for qc in range(QBC):
                        nc.tensor.matmul(
                            pvs[qc],
                            lhsT=V2[:, kt, h * 65:(h + 1) * 65],
                            rhs=ex[:, qc * 512:(qc + 1) * 512],
                            start=(kt == 0), stop=(kt == NKT - 1),
                        )
                # normalize into the stashed attnT half; project after h1
                for qc in range(QBC):
                    qq = qb * QBC + qc
                    # evacuate psum fast so the next block's PV can start
                    pvc = rpool.tile([65, 512], F32, tag="pvc")
                    nc.vector.tensor_copy(pvc, pvs[qc])
                    rc = rpool.tile([1, 512], F32, tag="rc")
                    nc.vector.reciprocal(rc, pvc[64:65, :])
                    scr = dpool.tile([1, 512], F32, tag="scr")
                    nc.sync.dma_start(out=scr, in_=rc)
                    bc = rpool.tile([DH, 512], F32, tag="bc")
                    nc.sync.dma_start(
                        out=bc,
                        in_=bass.AP(tensor=scr.tensor, offset=scr.offset,
                                    ap=[[0, DH]] + list(scr.ap)[1:]),
                    )
                    nc.vector.tensor_mul(ATT[qq][hs, :], pvc[0:DH, :], bc)
                    if h == 1:
                        for qs in range(NQS):
                            for ec in range(NEC):
                                po = opsum.tile([128, 512], F32, tag="pt_po")
                                nc.tensor.matmul(
                                    po,
                                    lhsT=ATT[qq][:, qs * 128:(qs + 1) * 128],
                                    rhs=wo_sb[:, ec * 512:(ec + 1) * 512],
                                    start=True, stop=True,
                                )
                                osb = apool.tile([128, 512], F32, tag="osb")
                                nc.vector.tensor_copy(osb, po)
                                nc.sync.dma_start(
                                    out=out[qq * 512 + qs * 128:
                                            qq * 512 + (qs + 1) * 128,
                                            ec * 512:(ec + 1) * 512],
                                    in_=osb,
                                )
    nc.finalize()
    return nc


def _get_nc(S=SEQ, mmdt="fp16"):
    key = (S, mmdt)
    if key not in _NC_CACHE:
        _NC_CACHE[key] = _build_nc(S=S, mmdt=mmdt)
    return _NC_CACHE[key]


def _make_in_maps(x, Wq, bq, Wk, bk, Wv, bv, Wo, npdt=np.float16):
    xT = np.ascontiguousarray(np.asarray(x, np.float32)[0].T.astype(npdt))
    Wq, Wk, Wv, Wo = (np.asarray(a, np.float32).astype(npdt) for a in (Wq, Wk, Wv, Wo))
    bq, bk, bv = (np.asarray(a, np.float32) for a in (bq, bk, bv))
    in_maps = []
    for c in range(N_CORES):
        sl = slice(c * HC, (c + 1) * HC)
        in_maps.append({
            "xT": xT,
            "wq": np.ascontiguousarray(Wq[:, sl]),
            "wk": np.ascontiguousarray(Wk[:, sl]),
            "wv": np.ascontiguousarray(Wv[:, sl]),
            "bq": np.ascontiguousarray(bq[sl]).reshape(HC, 1),
            "bk": np.ascontiguousarray(bk[sl]).reshape(HC, 1),
            "bv": np.ascontiguousarray(bv[sl]).reshape(HC, 1),
            "wo": np.ascontiguousarray(Wo[sl, :]),
        })
    return in_maps


def run(inputs, trace=False, mmdt="fp16"):
    """Run the kernel; returns (out [1,S,E] float32, BassKernelResults)."""
    from concourse.bass_utils import run_bass_kernel_spmd

    nc = _get_nc(mmdt=mmdt)
    npdt = np.float16 if mmdt == "fp16" else np.float32
    in_maps = _make_in_maps(
        inputs["x"], inputs["Wq"], inputs["bq"], inputs["Wk"], inputs["bk"],
        inputs["Wv"], inputs["bv"], inputs["Wo"], npdt=npdt,
    )
    res = run_bass_kernel_spmd(
        nc, in_maps, core_ids=list(range(N_CORES)), trace=trace
    )
    acc = np.zeros((SEQ, EMBED), np.float64)
    for c in range(N_CORES):
        acc += res.results[c]["out"]
    acc += np.asarray(inputs["bo"], np.float64)
    return acc.astype(np.float32).reshape(1, SEQ, EMBED), res


def kernel(x, Wq, bq, Wk, bk, Wv, bv, Wo, bo):
    out, _ = run(dict(x=x, Wq=Wq, bq=bq, Wk=Wk, bk=bk, Wv=Wv, bv=bv, Wo=Wo, bo=bo))
    return out


# revision 18
# speedup vs baseline: 1.4542x; 1.0535x over previous
"""TRN2 Bass/Tile kernel: 16-head MHA, B=1 S=4096 E=1024, head-sharded over 8 cores.

Sharding: tensor-parallel over heads. Core c owns heads {2c, 2c+1}: columns
[128c, 128(c+1)) of Wq/Wk/Wv (+bias slices) and rows [128c, 128(c+1)) of Wo.
Each core computes attention for its 2 heads and a partial out-projection
[S, E]; the host sums the 8 partials and adds bo (TP row-parallel unshard).

Per-core pipeline (all matmuls float32r, fp32 PSUM accumulate):
  A) QT/KT/VT [128ch, S] = W_c^T @ x^T   (lhsT=W-slice, rhs=xT tiles, +bias)
  B) V natural [S,ch] via PE transpose, packed [V_h|ones] for the l-sum trick
  C) scores^T [k,q] = K_h @ Q_h^T -> exp on ACT (scale=1/8) -> PV accumulate
     psum[65, q]: rows 0:64 = unnormalized attn^T, row 64 = softmax denom l
  D) recip(l) -> DMA partition-broadcast -> scale attn^T -> out-proj @ Wo_c
"""

import sys

for _p in ("/opt/trn_rl_repo", "/opt/pypackages"):
    if _p not in sys.path:
        sys.path.append(_p)

import numpy as np

EMBED = 1024
N_CORES = 8
HC = EMBED // N_CORES  # 128 channels = 2 heads per core
DH = 64                # head dim
SEQ = 4096

_NC_CACHE = {}


def _build_nc(S=SEQ, E=EMBED, mmdt="fp16"):
    from contextlib import ExitStack

    import concourse.bass as bass
    import concourse.mybir as mybir
    import concourse.tile as tile
    from concourse import bacc
    from concourse.masks import make_identity

    F32 = mybir.dt.float32
    MMDT = {"fp16": mybir.dt.float16, "f32r": mybir.dt.float32r,
            "fp32": mybir.dt.float32}[mmdt]

    ET = E // 128      # E-tiles of 128 (contraction for projections)
    NSC = S // 512     # 512-wide S chunks
    NKT = S // 128     # 128-wide key tiles
    KTG = 2            # key tiles per exp group ([128, 1024] psum staging)
    NG = NKT // KTG
    NQS = 512 // 128   # 128-q subtiles per chunk
    NEC = E // 512     # 512-wide E chunks of the out-projection

    def mm(ap):
        return ap

    nc = bacc.Bacc()
    xT = nc.declare_dram_parameter("xT", [E, S], MMDT, isOutput=False)
    wq = nc.declare_dram_parameter("wq", [E, HC], MMDT, isOutput=False)
    wk = nc.declare_dram_parameter("wk", [E, HC], MMDT, isOutput=False)
    wv = nc.declare_dram_parameter("wv", [E, HC], MMDT, isOutput=False)
    bq = nc.declare_dram_parameter("bq", [HC, 1], F32, isOutput=False)
    bk = nc.declare_dram_parameter("bk", [HC, 1], F32, isOutput=False)
    bv = nc.declare_dram_parameter("bv", [HC, 1], F32, isOutput=False)
    wo = nc.declare_dram_parameter("wo", [HC, E], MMDT, isOutput=False)
    out = nc.declare_dram_parameter("out", [S, E], F32, isOutput=True)

    with tile.TileContext(nc) as tc, ExitStack() as ctx:
        wpool = ctx.enter_context(tc.tile_pool(name="w", bufs=1))
        xpool = ctx.enter_context(tc.tile_pool(name="x", bufs=4))
        qkvpool = ctx.enter_context(tc.tile_pool(name="qkv", bufs=1))
        v2pool = ctx.enter_context(tc.tile_pool(name="v2", bufs=1))
        epool = ctx.enter_context(tc.tile_pool(name="e", bufs=3))
        apool = ctx.enter_context(tc.tile_pool(name="a", bufs=3))
        rpool = ctx.enter_context(tc.tile_pool(name="r", bufs=4))
        dpool = ctx.enter_context(tc.tile_pool(name="d", bufs=4, space="DRAM"))
        # PSUM: 4 banks staging + 2 PV accumulators + 2 out/transpose = 8
        spsum = ctx.enter_context(tc.tile_pool(name="sp", bufs=2, space="PSUM"))
        pvpsum = ctx.enter_context(tc.tile_pool(name="pv", bufs=2, space="PSUM"))
        opsum = ctx.enter_context(tc.tile_pool(name="op", bufs=2, space="PSUM"))

        # --- weights / constants ---
        w_sb = {}
        for name, src in (("wq", wq), ("wk", wk), ("wv", wv)):
            t = wpool.tile([128, ET, HC], MMDT, tag=name, name=name)
            nc.sync.dma_start(out=t, in_=src.rearrange("(a p) c -> p a c", p=128))
            w_sb[name] = t
        wo_sb = wpool.tile([HC, E], MMDT, tag="wo")
        nc.sync.dma_start(out=wo_sb, in_=wo[:, :])
        b_sb = {}
        for name, src in (("bq", bq), ("bk", bk), ("bv", bv)):
            t = wpool.tile([HC, 1], F32, tag=name, name=name)
            nc.sync.dma_start(out=t, in_=src[:, :])
            b_sb[name] = t
        ident = wpool.tile([128, 128], MMDT, tag="ident")
        make_identity(nc, ident)
        ones_sb = wpool.tile([128, 1], F32, tag="ones")
        nc.vector.memset(ones_sb, 1.0)

        # --- stage A: QT/KT/VT [128ch, S] chunked by 512 ---
        QT = [qkvpool.tile([HC, 512], MMDT, tag=f"qt{i}", name=f"qt{i}") for i in range(NSC)]
        KTZ = [[qkvpool.tile([HC, 512], MMDT, tag=f"ktz{h}_{i}", name=f"ktz{h}_{i}")
                for i in range(NSC)] for h in range(2)]
        VT = [qkvpool.tile([HC, 512], MMDT, tag=f"vt{i}", name=f"vt{i}") for i in range(NSC)]
        for sc in range(NSC):
            big1 = spsum.tile([128, 1024], F32, tag="big")
            big2 = opsum.tile([128, 512], F32, tag="pt_po")
            for et in range(ET):
                xt = xpool.tile([128, 512], MMDT, tag="xt")
                nc.sync.dma_start(
                    out=xt, in_=xT[et * 128:(et + 1) * 128, sc * 512:(sc + 1) * 512]
                )
                first, last = et == 0, et == ET - 1
                nc.tensor.matmul(big1[:, 0:512], lhsT=mm(w_sb["wq"][:, et, :]),
                                 rhs=mm(xt), start=first, stop=last)
                nc.tensor.matmul(big1[:, 512:1024], lhsT=mm(w_sb["wk"][:, et, :]),
                                 rhs=mm(xt), start=first, stop=last)
                nc.tensor.matmul(big2[:, 0:512], lhsT=mm(w_sb["wv"][:, et, :]),
                                 rhs=mm(xt), start=first, stop=last)
            nc.vector.tensor_scalar_add(QT[sc], big1[:, 0:512], b_sb["bq"])
            nc.vector.memset(KTZ[0][sc][DH:HC, :], 0.0)
            nc.vector.memset(KTZ[1][sc][0:DH, :], 0.0)
            nc.vector.tensor_scalar_add(KTZ[0][sc][0:DH, :], big1[0:DH, 512:1024],
                                        b_sb["bk"][0:DH, :])
            nc.vector.tensor_scalar_add(KTZ[1][sc][DH:HC, :], big1[DH:HC, 512:1024],
                                        b_sb["bk"][DH:HC, :])
            nc.vector.tensor_scalar_add(VT[sc], big2[:, 0:512], b_sb["bv"])

        # --- stage B: V2 [128k, NKT, 65*2] = [V_h0|ones|V_h1|ones] ---
        V2 = v2pool.tile([128, NKT, 195], MMDT, tag="V2")
        for kt in range(NKT):
            nc.vector.tensor_copy(V2[:, kt, 64:65], ones_sb)
            nc.vector.tensor_copy(V2[:, kt, 129:130], ones_sb)
            nc.vector.memset(V2[:, kt, 130:195], 0.0)
            pt = opsum.tile([128, 512], MMDT, tag="pt_po")
            nc.tensor.transpose(
                pt[:, 0:128], VT[kt // 4][:, (kt % 4) * 128:(kt % 4 + 1) * 128], ident
            )
            nc.vector.tensor_copy(V2[:, kt, 0:64], pt[:, 0:64])
            nc.vector.tensor_copy(V2[:, kt, 65:129], pt[:, 64:128])

        # --- stages C+D: kt-outer over 1024-q blocks; attnT halves stashed ---
        ATT = [apool.tile([128, 512], MMDT, tag=f"att{i}", name=f"att{i}")
               for i in range(NSC)]
        QBC = 2 if NSC % 2 == 0 else 1  # q-chunks per block
        for h in range(2):
            hs = slice(h * DH, (h + 1) * DH)
            for qb in range(NSC // QBC):
                pvs = [pvpsum.tile([128, 512], F32, tag="pv", name="pv")
                       for _ in range(QBC)]
                for kt in range(NKT):
                    sb = spsum.tile([128, 1024], F32, tag="big")
                    for qc in range(QBC):
                        qq = qb * QBC + qc
                        nc.tensor.matmul(
                            sb[:, qc * 512:(qc + 1) * 512],
                            lhsT=KTZ[h][kt // 4][:, (kt % 4) * 128:(kt % 4 + 1) * 128],
                            rhs=QT[qq][:, :],
                            start=True, stop=True,
                        )
                    ex = epool.tile([128, 1024], MMDT, tag="ex")
                    nc.scalar.activation(
                        ex[:, 0:QBC * 512], sb[:, 0:QBC * 512],
                        mybir.ActivationFunctionType.Exp, scale=0.125,
                    )
                    for qc in range(QBC):
                        nc.tensor.matmul(
                            pvs[qc],
                            lhsT=V2[:, kt, h * 65:h * 65 + 128],
                            rhs=ex[:, qc * 512:(qc + 1) * 512],
                            start=(kt == 0), stop=(kt == NKT - 1),
                        )
                # normalize into the stashed attnT half; project after h1
                for qc in range(QBC):
                    qq = qb * QBC + qc
                    # evacuate psum fast so the next block's PV can start
                    pvc = rpool.tile([65, 512], F32, tag="pvc")
                    nc.vector.tensor_copy(pvc, pvs[qc][0:65, :])
                    rc = rpool.tile([1, 512], F32, tag="rc")
                    nc.vector.reciprocal(rc, pvc[64:65, :])
                    scr = dpool.tile([1, 512], F32, tag="scr")
                    nc.sync.dma_start(out=scr, in_=rc)
                    bc = rpool.tile([DH, 512], F32, tag="bc")
                    nc.sync.dma_start(
                        out=bc,
                        in_=bass.AP(tensor=scr.tensor, offset=scr.offset,
                                    ap=[[0, DH]] + list(scr.ap)[1:]),
                    )
                    nc.vector.tensor_mul(ATT[qq][hs, :], pvc[0:DH, :], bc)
                    if h == 1:
                        for qs in range(NQS):
                            for ec in range(NEC):
                                po = opsum.tile([128, 512], F32, tag="pt_po")
                                nc.tensor.matmul(
                                    po,
                                    lhsT=ATT[qq][:, qs * 128:(qs + 1) * 128],
                                    rhs=wo_sb[:, ec * 512:(ec + 1) * 512],
                                    start=True, stop=True,
                                )
                                osb = apool.tile([128, 512], F32, tag="osb")
                                nc.vector.tensor_copy(osb, po)
                                nc.sync.dma_start(
                                    out=out[qq * 512 + qs * 128:
                                            qq * 512 + (qs + 1) * 128,
                                            ec * 512:(ec + 1) * 512],
                                    in_=osb,
                                )
    nc.finalize()
    return nc


def _get_nc(S=SEQ, mmdt="fp16"):
    key = (S, mmdt)
    if key not in _NC_CACHE:
        _NC_CACHE[key] = _build_nc(S=S, mmdt=mmdt)
    return _NC_CACHE[key]


def _make_in_maps(x, Wq, bq, Wk, bk, Wv, bv, Wo, npdt=np.float16):
    xT = np.ascontiguousarray(np.asarray(x, np.float32)[0].T.astype(npdt))
    Wq, Wk, Wv, Wo = (np.asarray(a, np.float32).astype(npdt) for a in (Wq, Wk, Wv, Wo))
    bq, bk, bv = (np.asarray(a, np.float32) for a in (bq, bk, bv))
    in_maps = []
    for c in range(N_CORES):
        sl = slice(c * HC, (c + 1) * HC)
        in_maps.append({
            "xT": xT,
            "wq": np.ascontiguousarray(Wq[:, sl]),
            "wk": np.ascontiguousarray(Wk[:, sl]),
            "wv": np.ascontiguousarray(Wv[:, sl]),
            "bq": np.ascontiguousarray(bq[sl]).reshape(HC, 1),
            "bk": np.ascontiguousarray(bk[sl]).reshape(HC, 1),
            "bv": np.ascontiguousarray(bv[sl]).reshape(HC, 1),
            "wo": np.ascontiguousarray(Wo[sl, :]),
        })
    return in_maps


def run(inputs, trace=False, mmdt="fp16"):
    """Run the kernel; returns (out [1,S,E] float32, BassKernelResults)."""
    from concourse.bass_utils import run_bass_kernel_spmd

    nc = _get_nc(mmdt=mmdt)
    npdt = np.float16 if mmdt == "fp16" else np.float32
    in_maps = _make_in_maps(
        inputs["x"], inputs["Wq"], inputs["bq"], inputs["Wk"], inputs["bk"],
        inputs["Wv"], inputs["bv"], inputs["Wo"], npdt=npdt,
    )
    res = run_bass_kernel_spmd(
        nc, in_maps, core_ids=list(range(N_CORES)), trace=trace
    )
    acc = np.zeros((SEQ, EMBED), np.float64)
    for c in range(N_CORES):
        acc += res.results[c]["out"]
    acc += np.asarray(inputs["bo"], np.float64)
    return acc.astype(np.float32).reshape(1, SEQ, EMBED), res


def kernel(x, Wq, bq, Wk, bk, Wv, bv, Wo, bo):
    out, _ = run(dict(x=x, Wq=Wq, bq=bq, Wk=Wk, bk=bk, Wv=Wv, bv=bv, Wo=Wo, bo=bo))
    return out
